# revision 4
# baseline (speedup 1.0000x reference)
"""Multi-head self-attention Bass/Tile kernel for Trainium2, 8 NeuronCores.

Problem: B=4, S=2048, D=1024, H=16 heads (HD=64), fp32, causal mask,
no padding.  y = softmax((xWq+bq)(xWk+bk)^T / 8 + mask) (xWv+bv) Wo + bo

Sharding (4-way batch x 2-way head-group):
  core c -> batch b = c//2, head group g = c%2 (heads 8g..8g+7).
  Each core computes its 8 heads' attention output and a PARTIAL
  out-projection y_partial = attn_out @ Wout[rows of its heads] (+ bout
  on g==0 cores only).  Host sums the two partials per batch.

v2 design (all matmuls bf16, f32 PSUM accumulation):
  Parity trick: K^T stays as natural head-PAIRS [128, S] in SBUF (rows
  0-63 = even head, 64-127 = odd head, never split or zero-padded).
  Q^T is stored per head [128, S] with the OPPOSITE 64 partitions ZERO,
  so the full-128-deep scores matmul K_pair^T.T @ Q_h contracts to
  exactly one head's scores.  Everything stays SBUF-resident (no DRAM
  round-trip for K).
  Exp batching: scores for groups of 3 k-chunks land in one 3-bank
  PSUM tile; a single ACT exp covers 1536 columns, amortizing the
  ~350-cycle ACT fixed overhead.
  Causal masking: only the [128,128] diagonal triangle gets a DVE
  mask-add; fully-masked columns of diagonal blocks are skipped by
  accumulating the AV matmul over a column sub-range.
  Softmax denominator: one-hot column 64+h of V_aug makes the AV
  matmul accumulate head h's denominator on PSUM row 64+h for free;
  reciprocal_approx_fast + PE broadcast normalizes at the end.
"""

import sys

if "/opt/trn_rl_repo" not in sys.path:
    sys.path.insert(0, "/opt/trn_rl_repo")

import ml_dtypes
import numpy as np

import concourse.bass as bass
import concourse.mybir as mybir
import concourse.tile as tile
from concourse import bacc
from concourse.bass_utils import run_bass_kernel_spmd

f32 = mybir.dt.float32
BF16 = mybir.dt.bfloat16
AF = mybir.ActivationFunctionType
OP = mybir.AluOpType

B, S, D, H = 4, 2048, 1024, 16
HD = D // H            # 64
P = 128
DC = D // P            # 8 contraction chunks for the projections
NPAIR = 4              # head pairs per core (8 local heads)
NST = S // 512         # 4 S-tiles of 512
NKC = S // P           # 16 k-chunks of 128
VW = HD + 8            # V_aug width: 64 V cols + 8 one-hot denominator cols
NEG = -1.0e30
EG = 3                 # k-chunks per exp group (3 PSUM banks)


def build_program():
    nc = bacc.Bacc("TRN2", target_bir_lowering=False, debug=False)

    xt_d = nc.dram_tensor("xT", [D, S], BF16, kind="ExternalInput")
    w_d = nc.dram_tensor("wqkv", [D, 3 * 512], BF16, kind="ExternalInput")
    b_d = nc.dram_tensor("bqkv", [3 * 512], f32, kind="ExternalInput")
    wo_d = nc.dram_tensor("wout", [512, D], BF16, kind="ExternalInput")
    bo_d = nc.dram_tensor("bout", [D], f32, kind="ExternalInput")
    cm_d = nc.dram_tensor("cmtri", [P, P], BF16, kind="ExternalInput")
    vm_d = nc.dram_tensor("vmask", [8, 8], BF16, kind="ExternalInput")
    sel_d = nc.dram_tensor("sel", [8, 8, HD], BF16, kind="ExternalInput")
    y_d = nc.dram_tensor("y", [S, D], f32, kind="ExternalOutput")

    from contextlib import ExitStack

    with tile.TileContext(nc) as tc, ExitStack() as _lp:
        _lp.enter_context(
            nc.allow_low_precision(reason="bf16 matmuls with f32 psum accumulation")
        )
        with tc.tile_pool(name="pers", bufs=1) as pers, \
             tc.tile_pool(name="consts", bufs=1) as consts:

            # ---- persistent activations ----
            # Q^T per head: live 64 rows at parity offset, other 64 rows ZERO
            q_all = pers.tile([P, 8, S], BF16, tag="q")
            # K^T natural head pairs (rows 0-63 even head, 64-127 odd head)
            kt_all = pers.tile([P, NPAIR, S], BF16, tag="kt")
            v_all = pers.tile([P, NKC, 8, VW], BF16, tag="v")
            attn_t = pers.tile([P, NPAIR, S], BF16, tag="attn")
            den = pers.tile([P, NST, 512], f32, tag="den")

            # ---- constants ----
            sel_sb = consts.tile([72, 8, HD], BF16, tag="sel")
            bq_sb = consts.tile([P, 12], f32, tag="bq")
            vb_sb = consts.tile([P, 512], f32, tag="vb")
            cm_sb = consts.tile([P, P], BF16, tag="cm")
            wo_sb = consts.tile([P, 4, D], BF16, tag="wout")
            bo_sb = consts.tile([P, D], f32, tag="bo")

            # zero the dead parity halves of q_all (DVE, off critical path)
            for h in range(8):
                dead = slice(HD, P) if h % 2 == 0 else slice(0, HD)
                nc.vector.memset(q_all[dead, h, :], 0.0)

            # ================= Stage A: QKV projections =================
            with tc.tile_pool(name="wqkvp", bufs=1) as wqkvp, \
                 tc.tile_pool(name="xtp", bufs=2) as xtp, \
                 tc.tile_pool(name="ps_mm", bufs=6, space="PSUM") as ps_mm:

                w_sb = wqkvp.tile([P, DC, 3 * 512], BF16, tag="wqkv")

                # x rows and weights on the fast Sync queue; small constants
                # ride the GpSimd (SWDGE) queue.
                xt_r = xt_d.rearrange("(dc p) s -> p dc s", p=P)
                xt0 = xtp.tile([P, DC, 512], BF16, tag="xt")
                nc.sync.dma_start(out=xt0[:], in_=xt_r[:, :, 0:512])
                for dc in range(DC):
                    nc.sync.dma_start(
                        out=w_sb[:, dc, :], in_=w_d[dc * P : (dc + 1) * P, :]
                    )
                nc.gpsimd.dma_start(out=bq_sb[:], in_=b_d.rearrange("(o p) -> p o", p=P))
                nc.gpsimd.dma_start(
                    out=vb_sb[:], in_=b_d[None, 1024:1536].to_broadcast([P, 512])
                )
                nc.gpsimd.dma_start(out=cm_sb[:], in_=cm_d[:])
                nc.gpsimd.dma_start(out=sel_sb[64:72, :, :], in_=sel_d[:])
                # one-hot denominator columns of V_aug: col 64+j = (j == h)
                for h in range(8):
                    nc.gpsimd.dma_start(
                        out=v_all[:, :, h, HD:VW],
                        in_=vm_d[None, None, h, :].to_broadcast([P, NKC, 8]),
                    )
                for pc in range(4):
                    nc.gpsimd.dma_start(
                        out=wo_sb[:, pc, :], in_=wo_d[pc * P : (pc + 1) * P, :]
                    )
                nc.gpsimd.dma_start(
                    out=bo_sb[:], in_=bo_d[None, :].to_broadcast([P, D])
                )

                xts = {0: xt0}

                def prefetch_xt(st):
                    t = xtp.tile([P, DC, 512], BF16, tag="xt", name=f"xt{st}")
                    nc.sync.dma_start(
                        out=t[:], in_=xt_r[:, :, st * 512 : (st + 1) * 512]
                    )
                    xts[st] = t

                for st in range(NST):
                    sl = slice(st * 512, (st + 1) * 512)
                    if st + 1 < NST:
                        prefetch_xt(st + 1)
                    xt = xts.pop(st)
                    # Q^T head-pair tiles -> parity-split per-head SBUF layout
                    for pr in range(NPAIR):
                        mm = ps_mm.tile([P, 512], f32, tag="mm")
                        for dc in range(DC):
                            nc.tensor.matmul(
                                mm[:],
                                w_sb[:, dc, pr * P : (pr + 1) * P],
                                xt[:, dc, :],
                                start=(dc == 0),
                                stop=(dc == DC - 1),
                            )
                        bcol = bq_sb[:, pr : pr + 1]
                        # even head: live rows 0-63; odd head: live rows 64-127
                        nc.scalar.activation(
                            out=q_all[0:HD, 2 * pr, sl],
                            in_=mm[0:HD, :],
                            func=AF.Identity,
                            bias=bcol[0:HD],
                        )
                        nc.scalar.activation(
                            out=q_all[HD:P, 2 * pr + 1, sl],
                            in_=mm[HD:P, :],
                            func=AF.Identity,
                            bias=bcol[HD:P],
                        )
                    # K^T head-pair tiles -> resident pair-packed SBUF layout
                    for pr in range(NPAIR):
                        mm = ps_mm.tile([P, 512], f32, tag="mm")
                        c0 = 512 + pr * P
                        for dc in range(DC):
                            nc.tensor.matmul(
                                mm[:],
                                w_sb[:, dc, c0 : c0 + P],
                                xt[:, dc, :],
                                start=(dc == 0),
                                stop=(dc == DC - 1),
                            )
                        nc.scalar.activation(
                            out=kt_all[:, pr, sl],
                            in_=mm[:],
                            func=AF.Identity,
                            bias=bq_sb[:, 4 + pr : 5 + pr],
                        )
                    # V: natural [S, hd] layout per 128-row chunk, all 8 heads
                    for sb in range(4):
                        mm = ps_mm.tile([P, 512], f32, tag="mm")
                        for dc in range(DC):
                            nc.tensor.matmul(
                                mm[:],
                                xt[:, dc, sb * P : (sb + 1) * P],
                                w_sb[:, dc, 1024:1536],
                                start=(dc == 0),
                                stop=(dc == DC - 1),
                            )
                        kc = st * 4 + sb
                        nc.vector.tensor_tensor(
                            v_all[:, kc, :, 0:HD],
                            mm[:].rearrange("p (h d) -> p h d", h=8),
                            vb_sb[:].rearrange("p (h d) -> p h d", h=8),
                            OP.add,
                        )

            # ================= Stage B: attention =================
            with tc.tile_pool(name="ppool", bufs=3) as ppool, \
                 tc.tile_pool(name="ps_sg", bufs=2, space="PSUM") as ps_sg, \
                 tc.tile_pool(name="ps_av", bufs=2, space="PSUM") as ps_av:
                # ---- B1: unnormalized attention + denominators ----
                for h in range(8):
                    pr, half = h // 2, h % 2
                    po = HD * half
                    for qt in range(NST):
                        q0 = qt * 512
                        nk = 4 * qt + 4
                        av = ps_av.tile([P, 512], f32, tag="avy")
                        for g0 in range(0, nk, EG):
                            gsz = min(EG, nk - g0)
                            sg = ps_sg.tile([P, EG * 512], f32, tag="sg")
                            for j in range(gsz):
                                kc = g0 + j
                                js = slice(j * 512, (j + 1) * 512)
                                nc.tensor.matmul(
                                    sg[:, js],
                                    kt_all[:, pr, kc * P : (kc + 1) * P],
                                    q_all[:, h, q0 : q0 + 512],
                                    start=True,
                                    stop=True,
                                )
                                m = kc - 4 * qt
                                if m >= 0:
                                    # triangular mask on the [128,128] diagonal
                                    nc.vector.tensor_tensor(
                                        sg[:, j * 512 + m * P : j * 512 + (m + 1) * P],
                                        sg[:, j * 512 + m * P : j * 512 + (m + 1) * P],
                                        cm_sb[:],
                                        OP.add,
                                    )
                            pt = ppool.tile([P, EG * 512], BF16, tag="pt")
                            nc.scalar.activation(
                                out=pt[:, 0 : gsz * 512],
                                in_=sg[:, 0 : gsz * 512],
                                func=AF.Exp,
                                scale=0.125,
                            )
                            for j in range(gsz):
                                kc = g0 + j
                                m = kc - 4 * qt
                                c0 = m * P if m > 0 else 0
                                nc.tensor.matmul(
                                    av[0:VW, c0:512],
                                    v_all[:, kc, h, :],
                                    pt[:, j * 512 + c0 : (j + 1) * 512],
                                    start=(kc == 0),
                                    stop=(kc == nk - 1),
                                    skip_group_check=True,
                                )
                        # park unnormalized output + denominator (row 64+h of
                        # av holds head h's denominator, other rows are zero,
                        # so accumulating the aligned [64:72] block is exact)
                        if h == 0:
                            nc.scalar.activation(
                                out=den[64:72, qt, :],
                                in_=av[64:72, :],
                                func=AF.Identity,
                            )
                        else:
                            nc.vector.tensor_tensor(
                                den[64:72, qt, :],
                                den[64:72, qt, :],
                                av[64:72, :],
                                OP.add,
                            )
                        if half == 0:
                            nc.scalar.activation(
                                out=attn_t[0:HD, pr, q0 : q0 + 512],
                                in_=av[0:HD, :],
                                func=AF.Identity,
                            )
                        else:
                            nc.vector.tensor_copy(
                                out=attn_t[HD:P, pr, q0 : q0 + 512],
                                in_=av[0:HD, :],
                            )

                # ---- B2: reciprocals (batched per q-tile) ----
                for qt in range(NST):
                    nc.vector.reciprocal(den[64:72, qt, :], den[64:72, qt, :])

                # ---- B3: broadcast reciprocals, normalize in place ----
                with tc.tile_pool(name="rbp", bufs=2) as rbp:
                    for qt in range(NST):
                        q0 = qt * 512
                        denb = rbp.tile([72, 512], BF16, tag="denb")
                        nc.scalar.activation(
                            out=denb[64:72, :],
                            in_=den[64:72, qt, :],
                            func=AF.Identity,
                        )
                        for h in range(8):
                            pr, half = h // 2, h % 2
                            po = HD * half
                            rb = ps_av.tile([P, 512], f32, tag="avy", name=f"rb{qt}_{h}")
                            nc.tensor.matmul(
                                rb[po : po + HD, :],
                                sel_sb[64:72, h, :],
                                denb[64:72, :],
                                start=True,
                                stop=True,
                            )
                            rbs = rbp.tile([P, 512], BF16, tag="rbs")
                            nc.scalar.activation(
                                out=rbs[po : po + HD, :],
                                in_=rb[po : po + HD, :],
                                func=AF.Identity,
                            )
                            nc.vector.tensor_tensor(
                                attn_t[po : po + HD, pr, q0 : q0 + 512],
                                attn_t[po : po + HD, pr, q0 : q0 + 512],
                                rbs[po : po + HD, :],
                                OP.mult,
                            )

                # ================= Stage C: out projection =================
                with tc.tile_pool(name="ystage", bufs=3) as ystage:
                    for qc in range(S // P):
                        q0 = qc * P
                        yt = ystage.tile([P, D], f32, tag="yt")
                        for nb in range(2):
                            yp = ps_av.tile([P, 512], f32, tag="avy",
                                            name=f"yp{qc}_{nb}")
                            for pc in range(4):
                                nc.tensor.matmul(
                                    yp[:],
                                    attn_t[:, pc, q0 : q0 + P],
                                    wo_sb[:, pc, nb * 512 : (nb + 1) * 512],
                                    start=(pc == 0),
                                    stop=(pc == 3),
                                )
                            nc.vector.tensor_tensor(
                                yt[:, nb * 512 : (nb + 1) * 512],
                                yp[:],
                                bo_sb[:, nb * 512 : (nb + 1) * 512],
                                OP.add,
                            )
                        nc.sync.dma_start(out=y_d[q0 : q0 + P, :], in_=yt[:])

    nc.finalize()
    return nc


_NC = None


def _get_nc():
    global _NC
    if _NC is None:
        _NC = build_program()
    return _NC


def _shard_inputs(x, causal_mask, padding_mask, W_qkv, b_qkv, W_out, b_out):
    bf16 = ml_dtypes.bfloat16
    x = np.ascontiguousarray(np.asarray(x, dtype=np.float32))
    W_qkv = np.asarray(W_qkv, dtype=np.float32)
    b_qkv = np.asarray(b_qkv, dtype=np.float32)
    W_out = np.asarray(W_out, dtype=np.float32)
    b_out = np.asarray(b_out, dtype=np.float32)
    causal_mask = np.asarray(causal_mask)
    padding_mask = np.asarray(padding_mask)

    assert not padding_mask.any(), "kernel assumes no padding"
    # additive triangle for the [128,128] diagonal block of scores^T[k, q]:
    # masked iff local k > local q
    cm = np.where(
        causal_mask[0:P, 0:P].T, np.float32(NEG), np.float32(0.0)
    ).astype(bf16)

    in_maps = []
    for c in range(8):
        b, g = c // 2, c % 2
        cols = slice(g * 512, (g + 1) * 512)
        w_slice = np.concatenate(
            [W_qkv[:, cols], W_qkv[:, 1024:2048][:, cols], W_qkv[:, 2048:3072][:, cols]],
            axis=1,
        )
        b_slice = np.concatenate(
            [b_qkv[cols], b_qkv[1024:2048][cols], b_qkv[2048:3072][cols]]
        )
        in_maps.append(
            {
                "xT": np.ascontiguousarray(x[b].T.astype(bf16)),
                "wqkv": np.ascontiguousarray(w_slice.astype(bf16)),
                "bqkv": np.ascontiguousarray(b_slice),
                "wout": np.ascontiguousarray(
                    W_out[g * 512 : (g + 1) * 512, :].astype(bf16)
                ),
                "bout": b_out if g == 0 else np.zeros_like(b_out),
                "cmtri": cm,
                "vmask": np.eye(8, dtype=np.float32).astype(bf16),
                "sel": np.repeat(
                    np.eye(8, dtype=np.float32)[:, :, None], HD, axis=2
                ).astype(bf16),
            }
        )
    return in_maps


def _run(in_maps, **kwargs):
    nc = _get_nc()
    return run_bass_kernel_spmd(nc, in_maps, core_ids=list(range(8)), **kwargs)


def kernel(**inputs):
    in_maps = _shard_inputs(**inputs)
    res = _run(in_maps)
    out = np.empty((B, S, D), dtype=np.float32)
    for b in range(B):
        out[b] = res.results[2 * b]["y"] + res.results[2 * b + 1]["y"]
    return out


def kernel_traced(**inputs):
    """Like kernel() but with NTFF tracing; returns (out, BassKernelResults)."""
    in_maps = _shard_inputs(**inputs)
    res = _run(in_maps, trace=True)
    out = np.empty((B, S, D), dtype=np.float32)
    for b in range(B):
        out[b] = res.results[2 * b]["y"] + res.results[2 * b + 1]["y"]
    return out, res


# revision 5
# speedup vs baseline: 1.3099x; 1.3099x over previous
"""Multi-head self-attention Bass/Tile kernel for Trainium2, 8 NeuronCores.

Problem: B=4, S=2048, D=1024, H=16 heads (HD=64), fp32, causal mask,
no padding.  y = softmax((xWq+bq)(xWk+bk)^T / 8 + mask) (xWv+bv) Wo + bo

Sharding (4-way batch x 2-way head-group):
  core c -> batch b = c//2, head group g = c%2 (heads 8g..8g+7).
  Each core computes its 8 heads' attention output and a PARTIAL
  out-projection y_partial = attn_out @ Wout[rows of its heads] (+ bout
  on g==0 cores only).  Host sums the two partials per batch.

v3 design (all matmuls bf16, f32 PSUM accumulation):
  Parity trick: K^T stays as natural head-PAIRS [128, S] in SBUF (rows
  0-63 = even head, 64-127 = odd head, never split or zero-padded).
  Q^T is stored per head [128, S] with the OPPOSITE 64 partitions ZERO,
  so the full-128-deep scores matmul K_pair^T.T @ Q_h contracts to
  exactly one head's scores.  Everything stays SBUF-resident.
  Exp batching: scores for groups of 3 k-chunks land in one 3-bank
  PSUM tile; a single ACT exp covers 1536 columns, amortizing the
  ~350-cycle ACT fixed overhead.
  Causal masking: only the [128,128] diagonal triangle gets a DVE
  mask-add; fully-masked columns of diagonal blocks are skipped by
  accumulating the AV matmul over a column sub-range.
  Softmax denominator: one-hot column 64+h of V_aug makes the AV
  matmul accumulate head h's denominator on PSUM row 64+h for free.
  DMA discipline: the shared DMA engine chokes on small/broadcast
  descriptors (they starved the weight loads for ~150us in v2), so all
  transfers are few and fat from host-prearranged layouts; one-hots
  and padding zeros are built on-chip with memsets; V/out-proj biases
  ride the PSUM accumulation as rank-1 matmuls (ones x bias_row).
"""

import sys

if "/opt/trn_rl_repo" not in sys.path:
    sys.path.insert(0, "/opt/trn_rl_repo")

import ml_dtypes
import numpy as np

import concourse.bass as bass
import concourse.mybir as mybir
import concourse.tile as tile
from concourse import bacc
from concourse.bass_utils import run_bass_kernel_spmd

f32 = mybir.dt.float32
BF16 = mybir.dt.bfloat16
AF = mybir.ActivationFunctionType
OP = mybir.AluOpType

B, S, D, H = 4, 2048, 1024, 16
HD = D // H            # 64
P = 128
DC = D // P            # 8 contraction chunks for the projections
NPAIR = 4              # head pairs per core (8 local heads)
NST = S // 512         # 4 S-tiles of 512
NKC = S // P           # 16 k-chunks of 128
VW = HD + 8            # V_aug width: 64 V cols + 8 one-hot denominator cols
NEG = -1.0e30
EG = 3                 # k-chunks per exp group (3 PSUM banks)


def build_program():
    nc = bacc.Bacc("TRN2", target_bir_lowering=False, debug=False)

    # host-prearranged layouts so every DMA is a fat contiguous transfer
    xt_d = nc.dram_tensor("xt", [P, NST, DC, 512], BF16, kind="ExternalInput")
    w_d = nc.dram_tensor("wqkv", [3, P, DC, 512], BF16, kind="ExternalInput")
    bq_d = nc.dram_tensor("bq12", [P, 12], f32, kind="ExternalInput")
    bv_d = nc.dram_tensor("bv", [1, 512], BF16, kind="ExternalInput")
    bo_d = nc.dram_tensor("bo", [1, D], BF16, kind="ExternalInput")
    wo_d = nc.dram_tensor("wout", [P, 4, D], BF16, kind="ExternalInput")
    cm_d = nc.dram_tensor("cmtri", [P, P], BF16, kind="ExternalInput")
    sel_d = nc.dram_tensor("sel", [8, 8, HD], BF16, kind="ExternalInput")
    y_d = nc.dram_tensor("y", [S, D], f32, kind="ExternalOutput")

    from contextlib import ExitStack

    with tile.TileContext(nc) as tc, ExitStack() as _lp:
        _lp.enter_context(
            nc.allow_low_precision(reason="bf16 matmuls with f32 psum accumulation")
        )
        with tc.tile_pool(name="pers", bufs=1) as pers, \
             tc.tile_pool(name="consts", bufs=1) as consts:

            # ---- persistent activations ----
            # Q^T per head: live 64 rows at parity offset, other 64 rows ZERO
            q_all = pers.tile([P, 8, S], BF16, tag="q")
            # K^T natural head pairs (rows 0-63 even head, 64-127 odd head)
            kt_all = pers.tile([P, NPAIR, S], BF16, tag="kt")
            v_all = pers.tile([P, NKC, 8, VW], BF16, tag="v")
            attn_t = pers.tile([P, NPAIR, S], BF16, tag="attn")
            den = pers.tile([P, NST, 512], f32, tag="den")

            # ---- constants ----
            sel_sb = consts.tile([72, 8, HD], BF16, tag="sel")
            bq_sb = consts.tile([P, 12], f32, tag="bq")
            bv_sb = consts.tile([1, 512], BF16, tag="bv")
            bo_sb = consts.tile([1, D], BF16, tag="bo")
            cm_sb = consts.tile([P, P], BF16, tag="cm")
            wo_sb = consts.tile([P, 4, D], BF16, tag="wout")
            ones = consts.tile([1, P], BF16, tag="ones")

            # ---- on-chip init (no DMA): zeros/one-hots via DVE memsets ----
            nc.vector.memset(ones[0:1, :], 1.0)
            for h in range(8):
                dead = slice(HD, P) if h % 2 == 0 else slice(0, HD)
                nc.vector.memset(q_all[dead, h, :], 0.0)
            # one-hot denominator columns of V_aug: col 64+j = (j == h)
            nc.vector.memset(v_all[:, :, :, HD:VW], 0.0)
            for h in range(8):
                nc.vector.memset(v_all[:, :, h, HD + h : HD + h + 1], 1.0)

            # ================= Stage A: QKV projections =================
            with tc.tile_pool(name="wqkvp", bufs=1) as wqkvp, \
                 tc.tile_pool(name="xtp", bufs=1) as xtp, \
                 tc.tile_pool(name="ps_mm", bufs=6, space="PSUM") as ps_mm:

                w_sb = wqkvp.tile([P, 3, DC, 512], BF16, tag="wqkv")
                xt = xtp.tile([P, NST, DC, 512], BF16, tag="xt")

                # big inputs on the Sync (HWDGE) queue, interleaved so the
                # first-needed chunk lands first; consts on the GpSimd queue
                nc.sync.dma_start(out=w_sb[:, 0], in_=w_d[0])
                nc.sync.dma_start(out=xt[:, 0], in_=xt_d[:, 0])
                nc.sync.dma_start(out=w_sb[:, 1], in_=w_d[1])
                nc.sync.dma_start(out=w_sb[:, 2], in_=w_d[2])
                for st in range(1, NST):
                    nc.sync.dma_start(out=xt[:, st], in_=xt_d[:, st])
                nc.gpsimd.dma_start(out=bq_sb[:], in_=bq_d[:])
                nc.gpsimd.dma_start(out=cm_sb[:], in_=cm_d[:])
                nc.gpsimd.dma_start(out=sel_sb[64:72, :, :], in_=sel_d[:])
                nc.gpsimd.dma_start(out=bv_sb[:], in_=bv_d[:])
                nc.gpsimd.dma_start(out=bo_sb[:], in_=bo_d[:])
                nc.gpsimd.dma_start(out=wo_sb[:], in_=wo_d[:])

                for st in range(NST):
                    sl = slice(st * 512, (st + 1) * 512)
                    # Q^T head-pair tiles -> parity-split per-head SBUF layout
                    for pr in range(NPAIR):
                        mm = ps_mm.tile([P, 512], f32, tag="mm")
                        for dc in range(DC):
                            nc.tensor.matmul(
                                mm[:],
                                w_sb[:, 0, dc, pr * P : (pr + 1) * P],
                                xt[:, st, dc, :],
                                start=(dc == 0),
                                stop=(dc == DC - 1),
                            )
                        bcol = bq_sb[:, pr : pr + 1]
                        # even head: live rows 0-63; odd head: live rows 64-127
                        nc.scalar.activation(
                            out=q_all[0:HD, 2 * pr, sl],
                            in_=mm[0:HD, :],
                            func=AF.Identity,
                            bias=bcol[0:HD],
                        )
                        nc.scalar.activation(
                            out=q_all[HD:P, 2 * pr + 1, sl],
                            in_=mm[HD:P, :],
                            func=AF.Identity,
                            bias=bcol[HD:P],
                        )
                    # K^T head-pair tiles -> resident pair-packed SBUF layout
                    for pr in range(NPAIR):
                        mm = ps_mm.tile([P, 512], f32, tag="mm")
                        for dc in range(DC):
                            nc.tensor.matmul(
                                mm[:],
                                w_sb[:, 1, dc, pr * P : (pr + 1) * P],
                                xt[:, st, dc, :],
                                start=(dc == 0),
                                stop=(dc == DC - 1),
                            )
                        nc.scalar.activation(
                            out=kt_all[:, pr, sl],
                            in_=mm[:],
                            func=AF.Identity,
                            bias=bq_sb[:, 4 + pr : 5 + pr],
                        )
                    # V: natural [S, hd] layout per 128-row chunk, all 8 heads;
                    # bias rides the accumulation as a rank-1 matmul
                    for sb in range(4):
                        mm = ps_mm.tile([P, 512], f32, tag="mm")
                        for dc in range(DC):
                            nc.tensor.matmul(
                                mm[:],
                                xt[:, st, dc, sb * P : (sb + 1) * P],
                                w_sb[:, 2, dc, :],
                                start=(dc == 0),
                                stop=False,
                            )
                        nc.tensor.matmul(
                            mm[:],
                            ones[0:1, :],
                            bv_sb[0:1, :],
                            start=False,
                            stop=True,
                        )
                        kc = st * 4 + sb
                        nc.vector.tensor_copy(
                            out=v_all[:, kc, :, 0:HD],
                            in_=mm[:].rearrange("p (h d) -> p h d", h=8),
                        )

            # ================= Stage B: attention =================
            with tc.tile_pool(name="ppool", bufs=3) as ppool, \
                 tc.tile_pool(name="ps_sg", bufs=2, space="PSUM") as ps_sg, \
                 tc.tile_pool(name="ps_av", bufs=2, space="PSUM") as ps_av:
                # ---- B1: unnormalized attention + denominators ----
                for h in range(8):
                    pr, half = h // 2, h % 2
                    po = HD * half
                    for qt in range(NST):
                        q0 = qt * 512
                        nk = 4 * qt + 4
                        av = ps_av.tile([P, 512], f32, tag="avy")
                        for g0 in range(0, nk, EG):
                            gsz = min(EG, nk - g0)
                            sg = ps_sg.tile([P, EG * 512], f32, tag="sg")
                            for j in range(gsz):
                                kc = g0 + j
                                js = slice(j * 512, (j + 1) * 512)
                                nc.tensor.matmul(
                                    sg[:, js],
                                    kt_all[:, pr, kc * P : (kc + 1) * P],
                                    q_all[:, h, q0 : q0 + 512],
                                    start=True,
                                    stop=True,
                                )
                                m = kc - 4 * qt
                                if m >= 0:
                                    # triangular mask on the [128,128] diagonal
                                    nc.vector.tensor_tensor(
                                        sg[:, j * 512 + m * P : j * 512 + (m + 1) * P],
                                        sg[:, j * 512 + m * P : j * 512 + (m + 1) * P],
                                        cm_sb[:],
                                        OP.add,
                                    )
                            pt = ppool.tile([P, EG * 512], BF16, tag="pt")
                            nc.scalar.activation(
                                out=pt[:, 0 : gsz * 512],
                                in_=sg[:, 0 : gsz * 512],
                                func=AF.Exp,
                                scale=0.125,
                            )
                            for j in range(gsz):
                                kc = g0 + j
                                m = kc - 4 * qt
                                c0 = m * P if m > 0 else 0
                                nc.tensor.matmul(
                                    av[0:VW, c0:512],
                                    v_all[:, kc, h, :],
                                    pt[:, j * 512 + c0 : (j + 1) * 512],
                                    start=(kc == 0),
                                    stop=(kc == nk - 1),
                                    skip_group_check=True,
                                )
                        # park unnormalized output + denominator (row 64+h of
                        # av holds head h's denominator, other rows are zero,
                        # so accumulating the aligned [64:72] block is exact)
                        if h == 0:
                            nc.scalar.activation(
                                out=den[64:72, qt, :],
                                in_=av[64:72, :],
                                func=AF.Identity,
                            )
                        else:
                            nc.vector.tensor_tensor(
                                den[64:72, qt, :],
                                den[64:72, qt, :],
                                av[64:72, :],
                                OP.add,
                            )
                        if half == 0:
                            nc.scalar.activation(
                                out=attn_t[0:HD, pr, q0 : q0 + 512],
                                in_=av[0:HD, :],
                                func=AF.Identity,
                            )
                        else:
                            nc.vector.tensor_copy(
                                out=attn_t[HD:P, pr, q0 : q0 + 512],
                                in_=av[0:HD, :],
                            )

                # ---- B2: reciprocals (batched per q-tile) ----
                for qt in range(NST):
                    nc.vector.reciprocal(den[64:72, qt, :], den[64:72, qt, :])

                # ---- B3: broadcast reciprocals, normalize in place ----
                with tc.tile_pool(name="rbp", bufs=2) as rbp:
                    for qt in range(NST):
                        q0 = qt * 512
                        denb = rbp.tile([72, 512], BF16, tag="denb")
                        nc.scalar.activation(
                            out=denb[64:72, :],
                            in_=den[64:72, qt, :],
                            func=AF.Identity,
                        )
                        for h in range(8):
                            pr, half = h // 2, h % 2
                            po = HD * half
                            rb = ps_av.tile([P, 512], f32, tag="avy", name=f"rb{qt}_{h}")
                            nc.tensor.matmul(
                                rb[po : po + HD, :],
                                sel_sb[64:72, h, :],
                                denb[64:72, :],
                                start=True,
                                stop=True,
                            )
                            rbs = rbp.tile([P, 512], BF16, tag="rbs")
                            nc.scalar.activation(
                                out=rbs[po : po + HD, :],
                                in_=rb[po : po + HD, :],
                                func=AF.Identity,
                            )
                            nc.vector.tensor_tensor(
                                attn_t[po : po + HD, pr, q0 : q0 + 512],
                                attn_t[po : po + HD, pr, q0 : q0 + 512],
                                rbs[po : po + HD, :],
                                OP.mult,
                            )

                # ================= Stage C: out projection =================
                with tc.tile_pool(name="ystage", bufs=3) as ystage:
                    for qc in range(S // P):
                        q0 = qc * P
                        yt = ystage.tile([P, D], f32, tag="yt")
                        for nb in range(2):
                            yp = ps_av.tile([P, 512], f32, tag="avy",
                                            name=f"yp{qc}_{nb}")
                            for pc in range(4):
                                nc.tensor.matmul(
                                    yp[:],
                                    attn_t[:, pc, q0 : q0 + P],
                                    wo_sb[:, pc, nb * 512 : (nb + 1) * 512],
                                    start=(pc == 0),
                                    stop=False,
                                )
                            nc.tensor.matmul(
                                yp[:],
                                ones[0:1, :],
                                bo_sb[0:1, nb * 512 : (nb + 1) * 512],
                                start=False,
                                stop=True,
                            )
                            nc.scalar.activation(
                                out=yt[:, nb * 512 : (nb + 1) * 512],
                                in_=yp[:],
                                func=AF.Identity,
                            )
                        nc.sync.dma_start(out=y_d[q0 : q0 + P, :], in_=yt[:])

    nc.finalize()
    return nc


_NC = None


def _get_nc():
    global _NC
    if _NC is None:
        _NC = build_program()
    return _NC


def _shard_inputs(x, causal_mask, padding_mask, W_qkv, b_qkv, W_out, b_out):
    bf16 = ml_dtypes.bfloat16
    x = np.ascontiguousarray(np.asarray(x, dtype=np.float32))
    W_qkv = np.asarray(W_qkv, dtype=np.float32)
    b_qkv = np.asarray(b_qkv, dtype=np.float32)
    W_out = np.asarray(W_out, dtype=np.float32)
    b_out = np.asarray(b_out, dtype=np.float32)
    causal_mask = np.asarray(causal_mask)
    padding_mask = np.asarray(padding_mask)

    assert not padding_mask.any(), "kernel assumes no padding"
    # additive triangle for the [128,128] diagonal block of scores^T[k, q]:
    # masked iff local k > local q
    cm = np.where(
        causal_mask[0:P, 0:P].T, np.float32(NEG), np.float32(0.0)
    ).astype(bf16)
    sel = np.repeat(np.eye(8, dtype=np.float32)[:, :, None], HD, axis=2).astype(bf16)

    in_maps = []
    for c in range(8):
        b, g = c // 2, c % 2
        cols = slice(g * 512, (g + 1) * 512)
        # [3, 128, 8, 512]: i-th projection, partition, dc chunk, column
        w3 = np.stack(
            [W_qkv[:, 1024 * i : 1024 * (i + 1)][:, cols] for i in range(3)]
        )  # [3, 1024, 512]
        w3 = np.ascontiguousarray(
            w3.reshape(3, DC, P, 512).transpose(0, 2, 1, 3).astype(bf16)
        )
        b3 = np.stack([b_qkv[1024 * i : 1024 * (i + 1)][cols] for i in range(3)])
        bq12 = np.ascontiguousarray(b3[0:2].reshape(8, P).T.astype(np.float32))
        bq12 = np.concatenate(
            [bq12, np.zeros((P, 4), np.float32)], axis=1
        )  # [128, 12]; V-bias columns unused
        xt = np.ascontiguousarray(
            x[b].T.reshape(DC, P, NST, 512).transpose(1, 2, 0, 3).astype(bf16)
        )
        wo = np.ascontiguousarray(
            W_out[g * 512 : (g + 1) * 512, :]
            .reshape(4, P, D)
            .transpose(1, 0, 2)
            .astype(bf16)
        )
        in_maps.append(
            {
                "xt": xt,
                "wqkv": w3,
                "bq12": bq12,
                "bv": np.ascontiguousarray(b3[2:3].astype(bf16)),
                "bo": (b_out if g == 0 else np.zeros_like(b_out))[None, :].astype(bf16),
                "wout": wo,
                "cmtri": cm,
                "sel": sel,
            }
        )
    return in_maps


def _run(in_maps, **kwargs):
    nc = _get_nc()
    return run_bass_kernel_spmd(nc, in_maps, core_ids=list(range(8)), **kwargs)


def kernel(**inputs):
    in_maps = _shard_inputs(**inputs)
    res = _run(in_maps)
    out = np.empty((B, S, D), dtype=np.float32)
    for b in range(B):
        out[b] = res.results[2 * b]["y"] + res.results[2 * b + 1]["y"]
    return out


def kernel_traced(**inputs):
    """Like kernel() but with NTFF tracing; returns (out, BassKernelResults)."""
    in_maps = _shard_inputs(**inputs)
    res = _run(in_maps, trace=True)
    out = np.empty((B, S, D), dtype=np.float32)
    for b in range(B):
        out[b] = res.results[2 * b]["y"] + res.results[2 * b + 1]["y"]
    return out, res


# revision 6
# speedup vs baseline: 1.3394x; 1.0225x over previous
"""Multi-head self-attention Bass/Tile kernel for Trainium2, 8 NeuronCores.

Problem: B=4, S=2048, D=1024, H=16 heads (HD=64), fp32, causal mask,
no padding.  y = softmax((xWq+bq)(xWk+bk)^T / 8 + mask) (xWv+bv) Wo + bo

Sharding (4-way batch x 2-way head-group):
  core c -> batch b = c//2, head group g = c%2 (heads 8g..8g+7).
  Each core computes its 8 heads' attention output and a PARTIAL
  out-projection y_partial = attn_out @ Wout[rows of its heads] (+ bout
  on g==0 cores only).  Host sums the two partials per batch.

v3 design (all matmuls bf16, f32 PSUM accumulation):
  Parity trick: K^T stays as natural head-PAIRS [128, S] in SBUF (rows
  0-63 = even head, 64-127 = odd head, never split or zero-padded).
  Q^T is stored per head [128, S] with the OPPOSITE 64 partitions ZERO,
  so the full-128-deep scores matmul K_pair^T.T @ Q_h contracts to
  exactly one head's scores.  Everything stays SBUF-resident.
  Exp batching: scores for groups of 3 k-chunks land in one 3-bank
  PSUM tile; a single ACT exp covers 1536 columns, amortizing the
  ~350-cycle ACT fixed overhead.
  Causal masking: only the [128,128] diagonal triangle gets a DVE
  mask-add; fully-masked columns of diagonal blocks are skipped by
  accumulating the AV matmul over a column sub-range.
  Softmax denominator: one-hot column 64+h of V_aug makes the AV
  matmul accumulate head h's denominator on PSUM row 64+h for free.
  DMA discipline: the shared DMA engine chokes on small/broadcast
  descriptors (they starved the weight loads for ~150us in v2), so all
  transfers are few and fat from host-prearranged layouts; one-hots
  and padding zeros are built on-chip with memsets; V/out-proj biases
  ride the PSUM accumulation as rank-1 matmuls (ones x bias_row).
"""

import sys

if "/opt/trn_rl_repo" not in sys.path:
    sys.path.insert(0, "/opt/trn_rl_repo")

import ml_dtypes
import numpy as np

import concourse.bass as bass
import concourse.mybir as mybir
import concourse.tile as tile
from concourse import bacc
from concourse.bass_utils import run_bass_kernel_spmd

f32 = mybir.dt.float32
BF16 = mybir.dt.bfloat16
AF = mybir.ActivationFunctionType
OP = mybir.AluOpType

B, S, D, H = 4, 2048, 1024, 16
HD = D // H            # 64
P = 128
DC = D // P            # 8 contraction chunks for the projections
NPAIR = 4              # head pairs per core (8 local heads)
NST = S // 512         # 4 S-tiles of 512
NKC = S // P           # 16 k-chunks of 128
VW = HD + 8            # V_aug width: 64 V cols + 8 one-hot denominator cols
NEG = -1.0e30
EG = 3                 # k-chunks per exp group (3 PSUM banks)


def build_program():
    nc = bacc.Bacc("TRN2", target_bir_lowering=False, debug=False)

    # host-prearranged layouts so every DMA is a fat contiguous transfer
    xt_d = nc.dram_tensor("xt", [P, NST, DC, 512], BF16, kind="ExternalInput")
    w_d = nc.dram_tensor("wqkv", [3, P, DC, 512], BF16, kind="ExternalInput")
    bq_d = nc.dram_tensor("bq12", [P, 12], f32, kind="ExternalInput")
    bv_d = nc.dram_tensor("bv", [1, 512], BF16, kind="ExternalInput")
    bo_d = nc.dram_tensor("bo", [1, D], BF16, kind="ExternalInput")
    wo_d = nc.dram_tensor("wout", [P, 4, D], BF16, kind="ExternalInput")
    cm_d = nc.dram_tensor("cmtri", [P, P], BF16, kind="ExternalInput")
    sel_d = nc.dram_tensor("sel", [8, 8, HD], BF16, kind="ExternalInput")
    y_d = nc.dram_tensor("y", [S, D], f32, kind="ExternalOutput")

    from contextlib import ExitStack

    with tile.TileContext(nc) as tc, ExitStack() as _lp:
        _lp.enter_context(
            nc.allow_low_precision(reason="bf16 matmuls with f32 psum accumulation")
        )
        with tc.tile_pool(name="pers", bufs=1) as pers, \
             tc.tile_pool(name="consts", bufs=1) as consts:

            # ---- persistent activations ----
            # Q^T per head: live 64 rows at parity offset, other 64 rows ZERO
            q_all = pers.tile([P, 8, S], BF16, tag="q")
            # K^T natural head pairs (rows 0-63 even head, 64-127 odd head)
            kt_all = pers.tile([P, NPAIR, S], BF16, tag="kt")
            v_all = pers.tile([P, NKC, 8, VW], BF16, tag="v")
            attn_t = pers.tile([P, NPAIR, S], BF16, tag="attn")
            den = pers.tile([P, NST, 512], f32, tag="den")

            # ---- constants ----
            sel_sb = consts.tile([72, 8, HD], BF16, tag="sel")
            bq_sb = consts.tile([P, 12], f32, tag="bq")
            bv_sb = consts.tile([1, 512], BF16, tag="bv")
            bo_sb = consts.tile([1, D], BF16, tag="bo")
            cm_sb = consts.tile([P, P], BF16, tag="cm")
            wo_sb = consts.tile([P, 4, D], BF16, tag="wout")
            ones = consts.tile([1, P], BF16, tag="ones")

            # ---- on-chip init (no DMA): zeros/one-hots via DVE memsets ----
            nc.vector.memset(ones[0:1, :], 1.0)
            for h in range(8):
                dead = slice(HD, P) if h % 2 == 0 else slice(0, HD)
                nc.vector.memset(q_all[dead, h, :], 0.0)
            # one-hot denominator columns of V_aug: col 64+j = (j == h)
            nc.vector.memset(v_all[:, :, :, HD:VW], 0.0)
            for h in range(8):
                nc.vector.memset(v_all[:, :, h, HD + h : HD + h + 1], 1.0)

            # ================= Stage A: QKV projections =================
            with tc.tile_pool(name="wqkvp", bufs=1) as wqkvp, \
                 tc.tile_pool(name="xtp", bufs=1) as xtp, \
                 tc.tile_pool(name="ps_mm", bufs=6, space="PSUM") as ps_mm:

                w_sb = wqkvp.tile([P, 3, DC, 512], BF16, tag="wqkv")
                xt = xtp.tile([P, NST, DC, 512], BF16, tag="xt")

                # big inputs on the Sync (HWDGE) queue, interleaved so the
                # first-needed chunk lands first; consts on the GpSimd queue
                nc.sync.dma_start(out=w_sb[:, 0], in_=w_d[0])
                nc.sync.dma_start(out=xt[:, 0], in_=xt_d[:, 0])
                nc.sync.dma_start(out=w_sb[:, 1], in_=w_d[1])
                nc.sync.dma_start(out=w_sb[:, 2], in_=w_d[2])
                for st in range(1, NST):
                    nc.sync.dma_start(out=xt[:, st], in_=xt_d[:, st])
                nc.gpsimd.dma_start(out=bq_sb[:], in_=bq_d[:])
                nc.gpsimd.dma_start(out=cm_sb[:], in_=cm_d[:])
                nc.gpsimd.dma_start(out=sel_sb[64:72, :, :], in_=sel_d[:])
                nc.gpsimd.dma_start(out=bv_sb[:], in_=bv_d[:])
                nc.gpsimd.dma_start(out=bo_sb[:], in_=bo_d[:])
                nc.gpsimd.dma_start(out=wo_sb[:], in_=wo_d[:])

                for st in range(NST):
                    sl = slice(st * 512, (st + 1) * 512)
                    # Q^T head-pair tiles -> parity-split per-head SBUF layout
                    for pr in range(NPAIR):
                        mm = ps_mm.tile([P, 512], f32, tag="mm")
                        for dc in range(DC):
                            nc.tensor.matmul(
                                mm[:],
                                w_sb[:, 0, dc, pr * P : (pr + 1) * P],
                                xt[:, st, dc, :],
                                start=(dc == 0),
                                stop=(dc == DC - 1),
                            )
                        bcol = bq_sb[:, pr : pr + 1]
                        # even head: live rows 0-63; odd head: live rows 64-127
                        nc.scalar.activation(
                            out=q_all[0:HD, 2 * pr, sl],
                            in_=mm[0:HD, :],
                            func=AF.Identity,
                            bias=bcol[0:HD],
                        )
                        nc.scalar.activation(
                            out=q_all[HD:P, 2 * pr + 1, sl],
                            in_=mm[HD:P, :],
                            func=AF.Identity,
                            bias=bcol[HD:P],
                        )
                    # K^T head-pair tiles -> resident pair-packed SBUF layout
                    for pr in range(NPAIR):
                        mm = ps_mm.tile([P, 512], f32, tag="mm")
                        for dc in range(DC):
                            nc.tensor.matmul(
                                mm[:],
                                w_sb[:, 1, dc, pr * P : (pr + 1) * P],
                                xt[:, st, dc, :],
                                start=(dc == 0),
                                stop=(dc == DC - 1),
                            )
                        nc.scalar.activation(
                            out=kt_all[:, pr, sl],
                            in_=mm[:],
                            func=AF.Identity,
                            bias=bq_sb[:, 4 + pr : 5 + pr],
                        )
                    # V: natural [S, hd] layout per 128-row chunk, all 8 heads;
                    # bias rides the accumulation as a rank-1 matmul
                    for sb in range(4):
                        mm = ps_mm.tile([P, 512], f32, tag="mm")
                        for dc in range(DC):
                            nc.tensor.matmul(
                                mm[:],
                                xt[:, st, dc, sb * P : (sb + 1) * P],
                                w_sb[:, 2, dc, :],
                                start=(dc == 0),
                                stop=False,
                            )
                        nc.tensor.matmul(
                            mm[:],
                            ones[0:1, :],
                            bv_sb[0:1, :],
                            start=False,
                            stop=True,
                        )
                        kc = st * 4 + sb
                        nc.vector.tensor_copy(
                            out=v_all[:, kc, :, 0:HD],
                            in_=mm[:].rearrange("p (h d) -> p h d", h=8),
                        )

            # ================= Stage B: attention =================
            with tc.tile_pool(name="ppool", bufs=3) as ppool, \
                 tc.tile_pool(name="ps_sg", bufs=2, space="PSUM") as ps_sg, \
                 tc.tile_pool(name="ps_av", bufs=2, space="PSUM") as ps_av:
                # ---- B1: unnormalized attention + denominators ----
                # The AV matmuls of each exp-group are emitted one group LATE
                # (software pipelining across (h, qt) unit boundaries) so the
                # in-order PE queue always has the next group's scores to run
                # while the ACT engine evaluates the current group's exp.
                pending = [None]

                def flush_pending():
                    if pending[0] is not None:
                        pending[0]()
                        pending[0] = None

                def emit_unit(h, qt):
                    pr, half = h // 2, h % 2
                    po = HD * half
                    q0 = qt * 512
                    nk = 4 * qt + 4
                    av = ps_av.tile([P, 512], f32, tag="avy", name=f"av{h}_{qt}")
                    for g0 in range(0, nk, EG):
                        gsz = min(EG, nk - g0)
                        sg = ps_sg.tile([P, EG * 512], f32, tag="sg",
                                        name=f"sg{h}_{qt}_{g0}")
                        for j in range(gsz):
                            kc = g0 + j
                            js = slice(j * 512, (j + 1) * 512)
                            nc.tensor.matmul(
                                sg[:, js],
                                kt_all[:, pr, kc * P : (kc + 1) * P],
                                q_all[:, h, q0 : q0 + 512],
                                start=True,
                                stop=True,
                            )
                            m = kc - 4 * qt
                            if m >= 0:
                                # triangular mask on the [128,128] diagonal
                                nc.vector.tensor_tensor(
                                    sg[:, j * 512 + m * P : j * 512 + (m + 1) * P],
                                    sg[:, j * 512 + m * P : j * 512 + (m + 1) * P],
                                    cm_sb[:],
                                    OP.add,
                                )
                        pt = ppool.tile([P, EG * 512], BF16, tag="pt",
                                        name=f"pt{h}_{qt}_{g0}")
                        nc.scalar.activation(
                            out=pt[:, 0 : gsz * 512],
                            in_=sg[:, 0 : gsz * 512],
                            func=AF.Exp,
                            scale=0.125,
                        )
                        flush_pending()

                        def av_group(h=h, qt=qt, g0=g0, gsz=gsz, pt=pt, av=av,
                                     last=(g0 + gsz == nk)):
                            nk_ = 4 * qt + 4
                            for j in range(gsz):
                                kc = g0 + j
                                m = kc - 4 * qt
                                c0 = m * P if m > 0 else 0
                                nc.tensor.matmul(
                                    av[0:VW, c0:512],
                                    v_all[:, kc, h, :],
                                    pt[:, j * 512 + c0 : (j + 1) * 512],
                                    start=(kc == 0),
                                    stop=(kc == nk_ - 1),
                                    skip_group_check=True,
                                )
                            if not last:
                                return
                            # park unnormalized output + denominator (row
                            # 64+h of av holds head h's denominator, other
                            # rows are zero, so accumulating the aligned
                            # [64:72] block is exact)
                            if h == 0:
                                nc.scalar.activation(
                                    out=den[64:72, qt, :],
                                    in_=av[64:72, :],
                                    func=AF.Identity,
                                )
                            else:
                                nc.vector.tensor_tensor(
                                    den[64:72, qt, :],
                                    den[64:72, qt, :],
                                    av[64:72, :],
                                    OP.add,
                                )
                            po_ = HD * (h % 2)
                            nc.vector.tensor_copy(
                                out=attn_t[po_ : po_ + HD, h // 2,
                                           qt * 512 : qt * 512 + 512],
                                in_=av[0:HD, :],
                            )

                        pending[0] = av_group

                for h in range(8):
                    for qt in range(NST):
                        emit_unit(h, qt)
                flush_pending()

                # ---- B2: reciprocals (batched per q-tile) ----
                for qt in range(NST):
                    nc.vector.reciprocal(den[64:72, qt, :], den[64:72, qt, :])

                # ---- B3: broadcast reciprocals, normalize in place ----
                with tc.tile_pool(name="rbp", bufs=2) as rbp:
                    for qt in range(NST):
                        q0 = qt * 512
                        denb = rbp.tile([72, 512], BF16, tag="denb")
                        nc.scalar.activation(
                            out=denb[64:72, :],
                            in_=den[64:72, qt, :],
                            func=AF.Identity,
                        )
                        for h in range(8):
                            pr, half = h // 2, h % 2
                            po = HD * half
                            rb = ps_av.tile([P, 512], f32, tag="avy", name=f"rb{qt}_{h}")
                            nc.tensor.matmul(
                                rb[po : po + HD, :],
                                sel_sb[64:72, h, :],
                                denb[64:72, :],
                                start=True,
                                stop=True,
                            )
                            rbs = rbp.tile([P, 512], BF16, tag="rbs")
                            nc.scalar.activation(
                                out=rbs[po : po + HD, :],
                                in_=rb[po : po + HD, :],
                                func=AF.Identity,
                            )
                            nc.vector.tensor_tensor(
                                attn_t[po : po + HD, pr, q0 : q0 + 512],
                                attn_t[po : po + HD, pr, q0 : q0 + 512],
                                rbs[po : po + HD, :],
                                OP.mult,
                            )

                # ================= Stage C: out projection =================
                with tc.tile_pool(name="ystage", bufs=3) as ystage:
                    for qc in range(S // P):
                        q0 = qc * P
                        yt = ystage.tile([P, D], f32, tag="yt")
                        for nb in range(2):
                            yp = ps_av.tile([P, 512], f32, tag="avy",
                                            name=f"yp{qc}_{nb}")
                            for pc in range(4):
                                nc.tensor.matmul(
                                    yp[:],
                                    attn_t[:, pc, q0 : q0 + P],
                                    wo_sb[:, pc, nb * 512 : (nb + 1) * 512],
                                    start=(pc == 0),
                                    stop=False,
                                )
                            nc.tensor.matmul(
                                yp[:],
                                ones[0:1, :],
                                bo_sb[0:1, nb * 512 : (nb + 1) * 512],
                                start=False,
                                stop=True,
                            )
                            nc.scalar.activation(
                                out=yt[:, nb * 512 : (nb + 1) * 512],
                                in_=yp[:],
                                func=AF.Identity,
                            )
                        nc.sync.dma_start(out=y_d[q0 : q0 + P, :], in_=yt[:])

    nc.finalize()
    return nc


_NC = None


def _get_nc():
    global _NC
    if _NC is None:
        _NC = build_program()
    return _NC


def _shard_inputs(x, causal_mask, padding_mask, W_qkv, b_qkv, W_out, b_out):
    bf16 = ml_dtypes.bfloat16
    x = np.ascontiguousarray(np.asarray(x, dtype=np.float32))
    W_qkv = np.asarray(W_qkv, dtype=np.float32)
    b_qkv = np.asarray(b_qkv, dtype=np.float32)
    W_out = np.asarray(W_out, dtype=np.float32)
    b_out = np.asarray(b_out, dtype=np.float32)
    causal_mask = np.asarray(causal_mask)
    padding_mask = np.asarray(padding_mask)

    assert not padding_mask.any(), "kernel assumes no padding"
    # additive triangle for the [128,128] diagonal block of scores^T[k, q]:
    # masked iff local k > local q
    cm = np.where(
        causal_mask[0:P, 0:P].T, np.float32(NEG), np.float32(0.0)
    ).astype(bf16)
    sel = np.repeat(np.eye(8, dtype=np.float32)[:, :, None], HD, axis=2).astype(bf16)

    in_maps = []
    for c in range(8):
        b, g = c // 2, c % 2
        cols = slice(g * 512, (g + 1) * 512)
        # [3, 128, 8, 512]: i-th projection, partition, dc chunk, column
        w3 = np.stack(
            [W_qkv[:, 1024 * i : 1024 * (i + 1)][:, cols] for i in range(3)]
        )  # [3, 1024, 512]
        w3 = np.ascontiguousarray(
            w3.reshape(3, DC, P, 512).transpose(0, 2, 1, 3).astype(bf16)
        )
        b3 = np.stack([b_qkv[1024 * i : 1024 * (i + 1)][cols] for i in range(3)])
        bq12 = np.ascontiguousarray(b3[0:2].reshape(8, P).T.astype(np.float32))
        bq12 = np.concatenate(
            [bq12, np.zeros((P, 4), np.float32)], axis=1
        )  # [128, 12]; V-bias columns unused
        xt = np.ascontiguousarray(
            x[b].T.reshape(DC, P, NST, 512).transpose(1, 2, 0, 3).astype(bf16)
        )
        wo = np.ascontiguousarray(
            W_out[g * 512 : (g + 1) * 512, :]
            .reshape(4, P, D)
            .transpose(1, 0, 2)
            .astype(bf16)
        )
        in_maps.append(
            {
                "xt": xt,
                "wqkv": w3,
                "bq12": bq12,
                "bv": np.ascontiguousarray(b3[2:3].astype(bf16)),
                "bo": (b_out if g == 0 else np.zeros_like(b_out))[None, :].astype(bf16),
                "wout": wo,
                "cmtri": cm,
                "sel": sel,
            }
        )
    return in_maps


def _run(in_maps, **kwargs):
    nc = _get_nc()
    return run_bass_kernel_spmd(nc, in_maps, core_ids=list(range(8)), **kwargs)


def kernel(**inputs):
    in_maps = _shard_inputs(**inputs)
    res = _run(in_maps)
    out = np.empty((B, S, D), dtype=np.float32)
    for b in range(B):
        out[b] = res.results[2 * b]["y"] + res.results[2 * b + 1]["y"]
    return out


def kernel_traced(**inputs):
    """Like kernel() but with NTFF tracing; returns (out, BassKernelResults)."""
    in_maps = _shard_inputs(**inputs)
    res = _run(in_maps, trace=True)
    out = np.empty((B, S, D), dtype=np.float32)
    for b in range(B):
        out[b] = res.results[2 * b]["y"] + res.results[2 * b + 1]["y"]
    return out, res


# revision 12
# speedup vs baseline: 1.5512x; 1.1582x over previous
"""Multi-head self-attention Bass/Tile kernel for Trainium2, 8 NeuronCores.

Problem: B=4, S=2048, D=1024, H=16 heads (HD=64), fp32, causal mask,
no padding.  y = softmax((xWq+bq)(xWk+bk)^T / 8 + mask) (xWv+bv) Wo + bo

Sharding (4-way batch x 2-way head-group):
  core c -> batch b = c//2, head group g = c%2 (heads 8g..8g+7).
  Each core computes its 8 heads' attention output and a PARTIAL
  out-projection y_partial = attn_out @ Wout[rows of its heads] (+ bout
  on g==0 cores only).  Host sums the two partials per batch.

v3 design (all matmuls bf16, f32 PSUM accumulation):
  Parity trick: K^T stays as natural head-PAIRS [128, S] in SBUF (rows
  0-63 = even head, 64-127 = odd head, never split or zero-padded).
  Q^T is stored per head [128, S] with the OPPOSITE 64 partitions ZERO,
  so the full-128-deep scores matmul K_pair^T.T @ Q_h contracts to
  exactly one head's scores.  Everything stays SBUF-resident.
  Exp batching: scores for groups of 3 k-chunks land in one 3-bank
  PSUM tile; a single ACT exp covers 1536 columns, amortizing the
  ~350-cycle ACT fixed overhead.
  Causal masking: only the [128,128] diagonal triangle gets a DVE
  mask-add; fully-masked columns of diagonal blocks are skipped by
  accumulating the AV matmul over a column sub-range.
  Softmax denominator: one-hot column 64+h of V_aug makes the AV
  matmul accumulate head h's denominator on PSUM row 64+h for free.
  DMA discipline: the shared DMA engine chokes on small/broadcast
  descriptors (they starved the weight loads for ~150us in v2), so all
  transfers are few and fat from host-prearranged layouts; one-hots
  and padding zeros are built on-chip with memsets; V/out-proj biases
  ride the PSUM accumulation as rank-1 matmuls (ones x bias_row).
"""

import sys

if "/opt/trn_rl_repo" not in sys.path:
    sys.path.insert(0, "/opt/trn_rl_repo")

import ml_dtypes
import numpy as np

import concourse.bass as bass
import concourse.mybir as mybir
import concourse.tile as tile
from concourse import bacc
from concourse.bass_utils import run_bass_kernel_spmd

f32 = mybir.dt.float32
BF16 = mybir.dt.bfloat16
F32R = mybir.dt.float32r
AF = mybir.ActivationFunctionType
OP = mybir.AluOpType

B, S, D, H = 4, 2048, 1024, 16
HD = D // H            # 64
P = 128
DC = D // P            # 8 contraction chunks for the projections
NPAIR = 4              # head pairs per core (8 local heads)
NST = S // 512         # 4 S-tiles of 512
NKC = S // P           # 16 k-chunks of 128
VW = HD + 8            # V_aug width: 64 V cols + 8 one-hot denominator cols
NEG = -1.0e30
EG = 2                 # k-chunks per exp group (2 PSUM banks)


def build_program():
    nc = bacc.Bacc("TRN2", target_bir_lowering=False, debug=False)

    # host-prearranged layouts so every DMA is a fat contiguous transfer
    xt_d = nc.dram_tensor("xt", [P, NST, DC, 512], BF16, kind="ExternalInput")
    w_d = nc.dram_tensor("wqkv", [3, P, DC, 512], BF16, kind="ExternalInput")
    bq_d = nc.dram_tensor("bq12", [P, 12], f32, kind="ExternalInput")
    bv_d = nc.dram_tensor("bv", [1, 512], BF16, kind="ExternalInput")
    bo_d = nc.dram_tensor("bo", [1, D], BF16, kind="ExternalInput")
    wo_d = nc.dram_tensor("wout", [P, 4, D], BF16, kind="ExternalInput")
    cm_d = nc.dram_tensor("cmtri", [P, P], BF16, kind="ExternalInput")
    sel_d = nc.dram_tensor("sel", [8, 8, HD], BF16, kind="ExternalInput")
    y_d = nc.dram_tensor("y", [S, D], f32, kind="ExternalOutput")

    from contextlib import ExitStack

    with tile.TileContext(nc) as tc, ExitStack() as _lp:
        _lp.enter_context(
            nc.allow_low_precision(reason="bf16 matmuls with f32 psum accumulation")
        )
        with tc.tile_pool(name="pers", bufs=1) as pers, \
             tc.tile_pool(name="consts", bufs=1) as consts:

            # ---- persistent activations ----
            # Q^T per head: live 64 rows at parity offset, other 64 rows ZERO
            q_all = pers.tile([P, 8, S], BF16, tag="q")
            # K^T natural head pairs (rows 0-63 even head, 64-127 odd head)
            kt_all = pers.tile([P, NPAIR, S], BF16, tag="kt")
            v_all = pers.tile([P, NKC, 8, VW], BF16, tag="v")
            attn_t = pers.tile([P, NPAIR, S], BF16, tag="attn")
            den = pers.tile([P, NST, 512], f32, tag="den")

            # ---- constants ----
            sel_sb = consts.tile([72, 8, HD], BF16, tag="sel")
            bq_sb = consts.tile([P, 12], f32, tag="bq")
            bv_sb = consts.tile([1, 512], BF16, tag="bv")
            bo_sb = consts.tile([1, D], BF16, tag="bo")
            cm_sb = consts.tile([P, P], BF16, tag="cm")
            wo_sb = consts.tile([P, 4, D], BF16, tag="wout")
            ones = consts.tile([1, P], BF16, tag="ones")

            # ---- on-chip init (no DMA): zeros/one-hots via DVE memsets ----
            nc.vector.memset(ones[0:1, :], 1.0)
            for h in range(8):
                dead = slice(HD, P) if h % 2 == 0 else slice(0, HD)
                nc.vector.memset(q_all[dead, h, :], 0.0)
            # one-hot denominator columns of V_aug: col 64+j = (j == h)
            nc.vector.memset(v_all[:, :, :, HD:VW], 0.0)
            for h in range(8):
                nc.vector.memset(v_all[:, :, h, HD + h : HD + h + 1], 1.0)

            # ================= Stage A: QKV projections =================
            with tc.tile_pool(name="wqkvp", bufs=1) as wqkvp, \
                 tc.tile_pool(name="xtp", bufs=1) as xtp, \
                 tc.tile_pool(name="ps_mm", bufs=6, space="PSUM") as ps_mm:

                w_sb = wqkvp.tile([P, 3, DC, 512], BF16, tag="wqkv")
                xt = xtp.tile([P, NST, DC, 512], BF16, tag="xt")

                # big inputs on the Sync (HWDGE) queue, interleaved so the
                # first-needed chunk lands first; consts on the GpSimd queue
                nc.sync.dma_start(out=w_sb[:, 0], in_=w_d[0])
                nc.sync.dma_start(out=xt[:, 0], in_=xt_d[:, 0])
                nc.sync.dma_start(out=w_sb[:, 1], in_=w_d[1])
                nc.sync.dma_start(out=w_sb[:, 2], in_=w_d[2])
                for st in range(1, NST):
                    nc.sync.dma_start(out=xt[:, st], in_=xt_d[:, st])
                nc.gpsimd.dma_start(out=bq_sb[:], in_=bq_d[:])
                nc.gpsimd.dma_start(out=cm_sb[:], in_=cm_d[:])
                nc.gpsimd.dma_start(out=sel_sb[64:72, :, :], in_=sel_d[:])
                nc.gpsimd.dma_start(out=bv_sb[:], in_=bv_d[:])
                nc.gpsimd.dma_start(out=bo_sb[:], in_=bo_d[:])
                nc.gpsimd.dma_start(out=wo_sb[:], in_=wo_d[:])

                for st in range(NST):
                    sl = slice(st * 512, (st + 1) * 512)
                    # Q^T head-pair tiles -> parity-split per-head SBUF layout
                    for pr in range(NPAIR):
                        mm = ps_mm.tile([P, 512], f32, tag="mm")
                        for dc in range(DC):
                            nc.tensor.matmul(
                                mm[:],
                                w_sb[:, 0, dc, pr * P : (pr + 1) * P],
                                xt[:, st, dc, :],
                                start=(dc == 0),
                                stop=(dc == DC - 1),
                            )
                        bcol = bq_sb[:, pr : pr + 1]
                        # even head: live rows 0-63; odd head: live rows 64-127
                        nc.scalar.activation(
                            out=q_all[0:HD, 2 * pr, sl],
                            in_=mm[0:HD, :],
                            func=AF.Identity,
                            bias=bcol[0:HD],
                        )
                        nc.scalar.activation(
                            out=q_all[HD:P, 2 * pr + 1, sl],
                            in_=mm[HD:P, :],
                            func=AF.Identity,
                            bias=bcol[HD:P],
                        )
                    # K^T head-pair tiles -> resident pair-packed SBUF layout
                    for pr in range(NPAIR):
                        mm = ps_mm.tile([P, 512], f32, tag="mm")
                        for dc in range(DC):
                            nc.tensor.matmul(
                                mm[:],
                                w_sb[:, 1, dc, pr * P : (pr + 1) * P],
                                xt[:, st, dc, :],
                                start=(dc == 0),
                                stop=(dc == DC - 1),
                            )
                        nc.scalar.activation(
                            out=kt_all[:, pr, sl],
                            in_=mm[:],
                            func=AF.Identity,
                            bias=bq_sb[:, 4 + pr : 5 + pr],
                        )
                    # V: natural [S, hd] layout per 128-row chunk, all 8 heads;
                    # bias rides the accumulation as a rank-1 matmul
                    for sb in range(4):
                        mm = ps_mm.tile([P, 512], f32, tag="mm")
                        for dc in range(DC):
                            nc.tensor.matmul(
                                mm[:],
                                xt[:, st, dc, sb * P : (sb + 1) * P],
                                w_sb[:, 2, dc, :],
                                start=(dc == 0),
                                stop=False,
                            )
                        nc.tensor.matmul(
                            mm[:],
                            ones[0:1, :],
                            bv_sb[0:1, :],
                            start=False,
                            stop=True,
                        )
                        kc = st * 4 + sb
                        nc.vector.tensor_copy(
                            out=v_all[:, kc, :, 0:HD],
                            in_=mm[:].rearrange("p (h d) -> p h d", h=8),
                        )

            # ================= Stage B: attention =================
            with tc.tile_pool(name="ppool", bufs=3) as ppool, \
                 tc.tile_pool(name="ystage", bufs=3) as ystage, \
                 tc.tile_pool(name="rbp", bufs=2) as rbp, \
                 tc.tile_pool(name="ps_sg", bufs=3, space="PSUM") as ps_sg, \
                 tc.tile_pool(name="ps_av", bufs=2, space="PSUM") as ps_av:
                # B1 runs qt-outer so each q-tile's denominators finish early;
                # B3+C for q-tile qt are emitted after B1(qt+1)'s units, which
                # keeps the in-order PE queue busy while the reciprocal chain
                # runs on the DVE.  AV matmuls are emitted two exp-groups late
                # (software pipelining across unit boundaries) so the PE
                # always has scores work while ACT evaluates exps.
                from collections import deque
                pending = deque()

                def flush_one():
                    if pending:
                        pending.popleft()()

                def emit_unit(h, qt):
                    pr, half = h // 2, h % 2
                    q0 = qt * 512
                    nk = 4 * qt + 4
                    av = ps_av.tile([P, 512], f32, tag="avy", name=f"av{h}_{qt}")
                    for g0 in range(0, nk, EG):
                        gsz = min(EG, nk - g0)
                        sg = ps_sg.tile([P, EG * 512], f32, tag="sg",
                                        name=f"sg{h}_{qt}_{g0}")
                        for j in range(gsz):
                            kc = g0 + j
                            js = slice(j * 512, (j + 1) * 512)
                            nc.tensor.matmul(
                                sg[:, js],
                                kt_all[:, pr, kc * P : (kc + 1) * P],
                                q_all[:, h, q0 : q0 + 512],
                                start=True,
                                stop=True,
                            )
                            m = kc - 4 * qt
                            if m >= 0:
                                # triangular mask on the [128,128] diagonal
                                nc.vector.tensor_tensor(
                                    sg[:, j * 512 + m * P : j * 512 + (m + 1) * P],
                                    sg[:, j * 512 + m * P : j * 512 + (m + 1) * P],
                                    cm_sb[:],
                                    OP.add,
                                )
                        pt = ppool.tile([P, EG * 512], BF16, tag="pt",
                                        name=f"pt{h}_{qt}_{g0}")
                        nc.scalar.activation(
                            out=pt[:, 0 : gsz * 512],
                            in_=sg[:, 0 : gsz * 512],
                            func=AF.Exp,
                            scale=0.125,
                        )
                        if len(pending) >= 2:
                            flush_one()

                        def av_group(h=h, qt=qt, g0=g0, gsz=gsz, pt=pt, av=av,
                                     last=(g0 + gsz == nk)):
                            nk_ = 4 * qt + 4
                            for j in range(gsz):
                                kc = g0 + j
                                m = kc - 4 * qt
                                c0 = m * P if m > 0 else 0
                                nc.tensor.matmul(
                                    av[0:VW, c0:512],
                                    v_all[:, kc, h, :],
                                    pt[:, j * 512 + c0 : (j + 1) * 512],
                                    start=(kc == 0),
                                    stop=(kc == nk_ - 1),
                                    skip_group_check=True,
                                )
                            if not last:
                                return
                            # park unnormalized output + denominator (row
                            # 64+h of av holds head h's denominator, other
                            # rows are zero, so accumulating the aligned
                            # [64:72] block is exact)
                            if h == 0:
                                nc.vector.tensor_copy(
                                    out=den[64:72, qt, :],
                                    in_=av[64:72, :],
                                )
                            else:
                                nc.vector.tensor_tensor(
                                    den[64:72, qt, :],
                                    den[64:72, qt, :],
                                    av[64:72, :],
                                    OP.add,
                                )
                            po_ = HD * (h % 2)
                            nc.vector.tensor_copy(
                                out=attn_t[po_ : po_ + HD, h // 2,
                                           qt * 512 : qt * 512 + 512],
                                in_=av[0:HD, :],
                            )

                        pending.append(av_group)

                def emit_b3(qt):
                    q0 = qt * 512
                    nc.vector.reciprocal(den[64:72, qt, :], den[64:72, qt, :])
                    denb = rbp.tile([72, 512], BF16, tag="denb", name=f"denb{qt}")
                    nc.scalar.activation(
                        out=denb[64:72, :],
                        in_=den[64:72, qt, :],
                        func=AF.Identity,
                    )
                    for h in range(8):
                        pr, half = h // 2, h % 2
                        po = HD * half
                        rb = ps_av.tile([P, 512], f32, tag="avy",
                                        name=f"rb{qt}_{h}")
                        nc.tensor.matmul(
                            rb[0:HD, :],
                            sel_sb[64:72, h, :],
                            denb[64:72, :],
                            start=True,
                            stop=True,
                        )
                        nc.vector.tensor_tensor(
                            attn_t[po : po + HD, pr, q0 : q0 + 512],
                            attn_t[po : po + HD, pr, q0 : q0 + 512],
                            rb[0:HD, :],
                            OP.mult,
                        )

                def emit_c(qt):
                    for qc in range(4 * qt, 4 * qt + 4):
                        q0 = qc * P
                        yt = ystage.tile([P, D], f32, tag="yt", name=f"yt{qc}")
                        for nb in range(2):
                            yp = ps_av.tile([P, 512], f32, tag="avy",
                                            name=f"yp{qc}_{nb}")
                            for pc in range(4):
                                nc.tensor.matmul(
                                    yp[:],
                                    attn_t[:, pc, q0 : q0 + P],
                                    wo_sb[:, pc, nb * 512 : (nb + 1) * 512],
                                    start=(pc == 0),
                                    stop=False,
                                )
                            nc.tensor.matmul(
                                yp[:],
                                ones[0:1, :],
                                bo_sb[0:1, nb * 512 : (nb + 1) * 512],
                                start=False,
                                stop=True,
                            )
                            nc.vector.tensor_copy(
                                out=yt[:, nb * 512 : (nb + 1) * 512],
                                in_=yp[:],
                            )
                        nc.sync.dma_start(out=y_d[q0 : q0 + P, :], in_=yt[:])

                for qt in range(NST):
                    for h in range(8):
                        emit_unit(h, qt)
                    if qt >= 1:
                        emit_b3(qt - 1)
                        emit_c(qt - 1)
                while pending:
                    flush_one()
                emit_b3(NST - 1)
                emit_c(NST - 1)

    nc.finalize()
    return nc


_NC = None


def _get_nc():
    global _NC
    if _NC is None:
        _NC = build_program()
    return _NC


def _shard_inputs(x, causal_mask, padding_mask, W_qkv, b_qkv, W_out, b_out):
    bf16 = ml_dtypes.bfloat16
    x = np.ascontiguousarray(np.asarray(x, dtype=np.float32))
    W_qkv = np.asarray(W_qkv, dtype=np.float32)
    b_qkv = np.asarray(b_qkv, dtype=np.float32)
    W_out = np.asarray(W_out, dtype=np.float32)
    b_out = np.asarray(b_out, dtype=np.float32)
    causal_mask = np.asarray(causal_mask)
    padding_mask = np.asarray(padding_mask)

    assert not padding_mask.any(), "kernel assumes no padding"
    # additive triangle for the [128,128] diagonal block of scores^T[k, q]:
    # masked iff local k > local q
    cm = np.where(
        causal_mask[0:P, 0:P].T, np.float32(NEG), np.float32(0.0)
    ).astype(bf16)
    sel = np.repeat(np.eye(8, dtype=np.float32)[:, :, None], HD, axis=2).astype(bf16)

    in_maps = []
    for c in range(8):
        b, g = c // 2, c % 2
        cols = slice(g * 512, (g + 1) * 512)
        # [3, 128, 8, 512]: i-th projection, partition, dc chunk, column
        w3 = np.stack(
            [W_qkv[:, 1024 * i : 1024 * (i + 1)][:, cols] for i in range(3)]
        )  # [3, 1024, 512]
        w3 = np.ascontiguousarray(
            w3.reshape(3, DC, P, 512).transpose(0, 2, 1, 3).astype(bf16)
        )
        b3 = np.stack([b_qkv[1024 * i : 1024 * (i + 1)][cols] for i in range(3)])
        bq12 = np.ascontiguousarray(b3[0:2].reshape(8, P).T.astype(np.float32))
        bq12 = np.concatenate(
            [bq12, np.zeros((P, 4), np.float32)], axis=1
        )  # [128, 12]; V-bias columns unused
        xt = np.ascontiguousarray(
            x[b].T.reshape(DC, P, NST, 512).transpose(1, 2, 0, 3).astype(bf16)
        )
        wo = np.ascontiguousarray(
            W_out[g * 512 : (g + 1) * 512, :]
            .reshape(4, P, D)
            .transpose(1, 0, 2)
            .astype(bf16)
        )
        in_maps.append(
            {
                "xt": xt,
                "wqkv": w3,
                "bq12": bq12,
                "bv": np.ascontiguousarray(b3[2:3].astype(bf16)),
                "bo": (b_out if g == 0 else np.zeros_like(b_out))[None, :].astype(bf16),
                "wout": wo,
                "cmtri": cm,
                "sel": sel,
            }
        )
    return in_maps


def _run(in_maps, **kwargs):
    nc = _get_nc()
    return run_bass_kernel_spmd(nc, in_maps, core_ids=list(range(8)), **kwargs)


def kernel(**inputs):
    in_maps = _shard_inputs(**inputs)
    res = _run(in_maps)
    out = np.empty((B, S, D), dtype=np.float32)
    for b in range(B):
        out[b] = res.results[2 * b]["y"] + res.results[2 * b + 1]["y"]
    return out


def kernel_traced(**inputs):
    """Like kernel() but with NTFF tracing; returns (out, BassKernelResults)."""
    in_maps = _shard_inputs(**inputs)
    res = _run(in_maps, trace=True)
    out = np.empty((B, S, D), dtype=np.float32)
    for b in range(B):
        out[b] = res.results[2 * b]["y"] + res.results[2 * b + 1]["y"]
    return out, res
